# revision 20
# baseline (speedup 1.0000x reference)
"""Trainium2 kernel for ChannelQuadLayer.

Per-pixel quadratic channel expansion + 1x1 conv:
    quad = x[:, ii] * x[:, jj]  (all 2080 upper-tri channel pairs)
    y    = concat([x, quad])    -> [B, 2144, H, W]
    out  = einsum('bchw,oc->bohw', y, fc_w)

Strategy (8 NeuronCores, batch-parallel, one sample per core):
  * The 2080 unordered channel pairs are exactly the cyclic diagonals
    d=0..32 of the 64-channel index ring: pairs {i, (i+d)%64}.
  * Host prepares 9 "rotation buffers" B_k = [roll(x,-t_k); roll(x,-u_k)]
    (128 partitions x 4096 pixels, bfloat16). A single elementwise
    multiply of two such buffers yields TWO complete cyclic diagonals
    (top half: diagonal t_j - t_i, bottom half: u_j - u_i). A difference
    cover produces all diagonals 1..32 in 16 multiplies; diagonal 0
    (squares) comes from one ScalarE Square op.
  * y-rows: 64 linear + 64 squares + 16*128 pair rows = 2176 = 17*128,
    an exact 17-chunk contraction. fc_w is permuted/padded to this row
    order on the host (duplicate pair rows get zero weight).
  * GEMM: out[256, 4096] = Wt[2176, 256]^T @ y[2176, 4096] on TensorE
    in bfloat16 (rel err ~3.5e-3, well under the 2e-2 gate; the bf16
    weight dtype enables the compiler's Fast Weight Load so the
    128-cycle LDWEIGHTS no longer serializes the matmul stream, unlike
    float32r), accumulating 17 chunks into fp32 PSUM, k-outer so each
    y chunk is consumed right after its producer. bf16 also gives the
    DVE 2x perf mode for the pair products and halves all DMA.
  * All 9 rotation buffers are packed into ONE DRAM tensor in
    pass-blocked, first-use order, so each pixel pass is a single
    dma_start on SyncE (DMA issues cost ~0.6us of sequencer time each
    and, queued behind compute, caused PE starvation). Outputs
    similarly drain to one staging tile and one DMA per pass.
  * Output leaves the chip as bf16 [128, 2, PIX]; the host upcasts and
    reorders - host work is untimed.
  * Pixel passes [256, 512, 1024, 1024, 1024, 256]: small first pass
    fills the pipeline fast, small last pass shortens the tail. Chunk 0
    (ScalarE, gated on the 1.3us activation-table load) is reordered to
    the END of each pass's accumulation.
  * A few junk matmuls on a memset tile warm the PE clock gate (HAM)
    while the first DMAs are in flight.
"""

import sys

sys.path.insert(0, "/opt/trn_rl_repo")

import numpy as np
import ml_dtypes

import concourse.bass as bass
import concourse.tile as tile
from concourse import bacc, mybir
from concourse.bass_utils import run_bass_kernel_spmd

B, C, H, W = 8, 64, 64, 64
PIX = H * W  # 4096
OUT = 256
NCORES = 8

# rotation difference cover: ops (i,j) give diagonals D(t_j-t_i) (top half)
# and D(u_j-u_i) (bottom half); together exactly {1..32}.
T_ROT = [0, 8, 22, 24, 42, 48, 49, 57, 60]
U_ROT = [0, 59, 16, 38, 55, 22, 30, 54, 35]
OPS = [(1, 3), (2, 3), (1, 4), (2, 4), (3, 4), (4, 5), (1, 6), (2, 6),
       (6, 7), (0, 7), (4, 7), (5, 7), (2, 8), (3, 8), (5, 8), (6, 8)]
NB = len(T_ROT)        # 9 rotation buffers
KCH = 1 + len(OPS)     # 17 contraction chunks of 128 rows
PASS_FD = [256, 512, 1024, 1024, 1024, 256]
assert sum(PASS_FD) == PIX
NPASS = len(PASS_FD)
# chunk order within a pass: chunk 0 (ScalarE path, gated on the 1.3us
# activation-table load) goes last so TT-produced chunks start the
# accumulation immediately.
KORDER = list(range(1, KCH)) + [0]
# packed order of the rotation buffers = first-use order under KORDER,
# so pass 0 can be split into two DMAs with the urgent half first.
BUF_ORDER = [1, 3, 2, 4, 5, 6, 7, 0, 8]
POS = {b: p for p, b in enumerate(BUF_ORDER)}
NSPLIT = 5  # pass-0 buffers in the first (urgent) DMA

F32 = mybir.dt.float32
BF16 = mybir.dt.bfloat16


def row_pairs():
    """Channel pair (c1, c2) for every global y row, or ('lin', c)."""
    rows = []
    for p in range(128):  # chunk 0
        rows.append(("lin", p) if p < 64 else (p - 64, p - 64))
    for (i, j) in OPS:
        for p in range(128):
            if p < 64:
                c1, c2 = (p + T_ROT[i]) % 64, (p + T_ROT[j]) % 64
            else:
                c1, c2 = (p - 64 + U_ROT[i]) % 64, (p - 64 + U_ROT[j]) % 64
            rows.append((min(c1, c2), max(c1, c2)))
    return rows


def build_wt(fc_w):
    """Permute fc_w [OUT, 2144] into Wt [KCH, 128, OUT] matching y rows."""
    ii, jj = np.triu_indices(C)
    pair2col = {(a, b): C + k for k, (a, b) in enumerate(zip(ii, jj))}
    wt = np.zeros((KCH * 128, OUT), np.float32)
    seen = set()
    for g, r in enumerate(row_pairs()):
        if r[0] == "lin":
            wt[g] = fc_w[:, r[1]]
        elif r not in seen:
            seen.add(r)
            wt[g] = fc_w[:, pair2col[r]]
    assert len(seen) == C * (C + 1) // 2
    return np.ascontiguousarray(wt.reshape(KCH, 128, OUT))


_cached = None


def _build_module():
    global _cached
    if _cached is not None:
        return _cached
    nc = bacc.Bacc("TRN2", target_bir_lowering=False, debug=False,
                   num_devices=NCORES)
    # all 9 rotation buffers, pass-blocked + packed in BUF_ORDER:
    # pass p occupies columns [9*off_p, 9*off_p + 9*FD_p), buffer j (packed)
    # at [9*off_p + j*FD_p, 9*off_p + (j+1)*FD_p).
    ball_d = nc.dram_tensor("ball", [128, NB * PIX], BF16,
                            kind="ExternalInput")
    # weight matrix, partition-major so DMA rows are contiguous
    wt_d = nc.dram_tensor("wt", [128, KCH * OUT], BF16, kind="ExternalInput")
    out_d = nc.dram_tensor("out", [128, 2, PIX], BF16, kind="ExternalOutput")

    with tile.TileContext(nc) as tc:
        with tc.tile_pool(name="wt", bufs=1) as wt_pool, \
             tc.tile_pool(name="bsrc", bufs=2) as b_pool, \
             tc.tile_pool(name="y", bufs=8) as y_pool, \
             tc.tile_pool(name="ostage", bufs=3) as o_pool, \
             tc.tile_pool(name="psum", bufs=8, space="PSUM") as ps_pool:

            wt_t = wt_pool.tile([128, KCH * OUT], BF16, name="wtt")

            # PE warm-up: junk matmuls on a memset tile keep TensorE
            # CONTINUOUSLY busy from ~5us (right after the framework
            # preamble) until the first real inputs land (~12us), so the
            # HAM clock-gate reaches 8/8 and the real stream starts warm.
            # A sparse cold start is worse than a late dense one: cold
            # matmuls run at 1.2GHz and idle gaps keep resetting HAM's
            # 3.4us busy window.
            wu = wt_pool.tile([128, 640], BF16, name="warmup")
            nc.vector.memset(wu[:, :], 0.0)
            ps_w = ps_pool.tile([128, 512], F32, tag="ps", name="warm_ps")
            for r in range(9):
                nc.tensor.matmul(ps_w[:, :], wu[:, 0:128], wu[:, 128:640],
                                 start=True, stop=True)

            off = 0
            for ps, FD in enumerate(PASS_FD):
                NT = max(1, FD // 512)
                NW = min(512, FD)  # matmul free width
                blk = NB * off     # start column of this pass's block
                ball_t = b_pool.tile([128, NB * 1024], BF16, tag="ball",
                                     name=f"ball_{ps}")
                if ps == 0:
                    # three queues in parallel, most-urgent first per queue.
                    # GpSimd's SWDGE queue starts ~3.5us later than the
                    # HWDGE ones, so it only gets weights needed mid-pass;
                    # Scalar's DMAs are issued before any of its compute so
                    # they can't block on drains.
                    nc.scalar.dma_start(wt_t[:, OUT:9 * OUT],
                                        wt_d.ap()[:, OUT:9 * OUT])
                    nc.gpsimd.dma_start(wt_t[:, 9 * OUT:13 * OUT],
                                        wt_d.ap()[:, 9 * OUT:13 * OUT])
                    nc.sync.dma_start(
                        ball_t[:, :NSPLIT * FD],
                        ball_d.ap()[:, blk:blk + NSPLIT * FD])
                    nc.sync.dma_start(
                        ball_t[:, NSPLIT * FD:NB * FD],
                        ball_d.ap()[:, blk + NSPLIT * FD:blk + NB * FD])
                    nc.sync.dma_start(wt_t[:, 13 * OUT:],
                                      wt_d.ap()[:, 13 * OUT:])
                    nc.sync.dma_start(wt_t[:, :OUT], wt_d.ap()[:, :OUT])
                else:
                    # split every pass across two DMA queues so neither
                    # (~180GB/s each) gates a pass boundary
                    nc.sync.dma_start(
                        ball_t[:, :NSPLIT * FD],
                        ball_d.ap()[:, blk:blk + NSPLIT * FD])
                    nc.gpsimd.dma_start(
                        ball_t[:, NSPLIT * FD:NB * FD],
                        ball_d.ap()[:, blk + NSPLIT * FD:blk + NB * FD])

                def bsl(i):
                    return ball_t[:, POS[i] * FD:(POS[i] + 1) * FD]

                psum = [ps_pool.tile([128, 512], F32, tag="ps",
                                     name=f"ps{ps}_{g}")
                        for g in range(2 * NT)]

                for k in KORDER:
                    yk = y_pool.tile([128, 1024], BF16, tag="y",
                                     name=f"y{ps}_{k}")
                    if k == 0:
                        # linear rows + squares from the resident b0 slice
                        b0 = bsl(0)
                        nc.scalar.activation(
                            yk[0:64, :FD], b0[0:64, :],
                            mybir.ActivationFunctionType.Identity)
                        nc.scalar.activation(
                            yk[64:128, :FD], b0[64:128, :],
                            mybir.ActivationFunctionType.Square)
                    else:
                        i, j = OPS[k - 1]
                        nc.vector.tensor_mul(yk[:, :FD], bsl(i), bsl(j))
                    for m in range(2):
                        lhsT = wt_t[:, k * OUT + m * 128:k * OUT + (m + 1) * 128]
                        for n in range(NT):
                            nc.tensor.matmul(
                                psum[m * NT + n][:, :NW],
                                lhsT,
                                yk[:, n * NW:(n + 1) * NW],
                                start=(k == KORDER[0]), stop=(k == KORDER[-1]))

                last = ps == NPASS - 1
                ot = o_pool.tile([128, 2048], BF16, tag="ostage",
                                 name=f"o{ps}")
                for m in range(2):
                    for n in range(NT):
                        src = psum[m * NT + n][:, :NW]
                        dst = ot[:, m * FD + n * NW:m * FD + (n + 1) * NW]
                        if last and m == 1:
                            # tail: drain half the PSUM on the idle VectorE
                            nc.vector.tensor_copy(dst, src)
                        else:
                            nc.scalar.activation(
                                dst, src, mybir.ActivationFunctionType.Identity)
                eng = nc.sync if last else nc.scalar
                eng.dma_start(out_d.ap()[:, :, off:off + FD], ot[:, :2 * FD])
                off += FD
    nc.compile()
    _cached = nc
    return nc


def make_in_maps(x, wt):
    # [KCH, 128, OUT] -> [128, KCH*OUT], bf16
    wtp = np.ascontiguousarray(
        wt.transpose(1, 0, 2).reshape(128, KCH * OUT)).astype(ml_dtypes.bfloat16)
    in_maps = []
    for b in range(B):
        xc = np.ascontiguousarray(
            np.asarray(x[b], np.float32).reshape(C, PIX)).astype(ml_dtypes.bfloat16)
        bufs = [np.concatenate(
            [np.roll(xc, -T_ROT[i], axis=0), np.roll(xc, -U_ROT[i], axis=0)])
            for i in range(NB)]
        ball = np.empty((128, NB * PIX), ml_dtypes.bfloat16)
        o = 0
        for FD in PASS_FD:
            for j, bi in enumerate(BUF_ORDER):
                ball[:, NB * o + j * FD:NB * o + (j + 1) * FD] = \
                    bufs[bi][:, o:o + FD]
            o += FD
        in_maps.append({"wt": wtp, "ball": ball})
    return in_maps


def assemble_out(res):
    outs = []
    for b in range(B):
        o = np.asarray(res.results[b]["out"]).astype(np.float32)  # [128,2,PIX]
        outs.append(o.transpose(1, 0, 2).reshape(OUT, H, W))
    return np.stack(outs)


def kernel(x, fc_w):
    x = np.asarray(x, dtype=np.float32)
    fc_w = np.asarray(fc_w, dtype=np.float32)
    nc = _build_module()
    wt = build_wt(fc_w)
    res = run_bass_kernel_spmd(nc, make_in_maps(x, wt), list(range(NCORES)))
    return assemble_out(res)


# revision 22
# speedup vs baseline: 1.1186x; 1.1186x over previous
"""Trainium2 kernel for ChannelQuadLayer.

Per-pixel quadratic channel expansion + 1x1 conv:
    quad = x[:, ii] * x[:, jj]  (all 2080 upper-tri channel pairs)
    y    = concat([x, quad])    -> [B, 2144, H, W]
    out  = einsum('bchw,oc->bohw', y, fc_w)

Strategy (8 NeuronCores, batch-parallel, one sample per core):
  * The 2080 unordered channel pairs are exactly the cyclic diagonals
    d=0..32 of the 64-channel index ring: pairs {i, (i+d)%64}.
  * Host prepares 9 "rotation buffers" B_k = [roll(x,-t_k); roll(x,-u_k)]
    (128 partitions x 4096 pixels, bfloat16). A single elementwise
    multiply of two such buffers yields TWO complete cyclic diagonals
    (top half: diagonal t_j - t_i, bottom half: u_j - u_i). A difference
    cover produces all diagonals 1..32 in 16 multiplies; diagonal 0
    (squares) comes from one ScalarE Square op.
  * y-rows: 64 linear + 64 squares + 16*128 pair rows = 2176 = 17*128,
    an exact 17-chunk contraction. fc_w is permuted/padded to this row
    order on the host (duplicate pair rows get zero weight).
  * GEMM: out[256, 4096] = Wt[2176, 256]^T @ y[2176, 4096] on TensorE
    in bfloat16 (rel err ~3.5e-3, well under the 2e-2 gate; the bf16
    weight dtype enables the compiler's Fast Weight Load so the
    128-cycle LDWEIGHTS no longer serializes the matmul stream, unlike
    float32r), accumulating 17 chunks into fp32 PSUM, k-outer so each
    y chunk is consumed right after its producer. bf16 also gives the
    DVE 2x perf mode for the pair products and halves all DMA.
  * All 9 rotation buffers are packed into ONE DRAM tensor in
    pass-blocked, first-use order, so each pixel pass is a single
    dma_start on SyncE (DMA issues cost ~0.6us of sequencer time each
    and, queued behind compute, caused PE starvation). Outputs
    similarly drain to one staging tile and one DMA per pass.
  * Output leaves the chip as bf16 [128, 2, PIX]; the host upcasts and
    reorders - host work is untimed.
  * Pixel passes [256, 512, 1024, 1024, 1024, 256]: small first pass
    fills the pipeline fast, small last pass shortens the tail. Chunk 0
    (ScalarE, gated on the 1.3us activation-table load) is reordered to
    the END of each pass's accumulation.
  * A few junk matmuls on a memset tile warm the PE clock gate (HAM)
    while the first DMAs are in flight.
"""

import sys

sys.path.insert(0, "/opt/trn_rl_repo")

import numpy as np
import ml_dtypes

import concourse.bass as bass
import concourse.tile as tile
from concourse import bacc, mybir
from concourse.bass_utils import run_bass_kernel_spmd

B, C, H, W = 8, 64, 64, 64
PIX = H * W  # 4096
OUT = 256
NCORES = 8

# rotation difference cover: ops (i,j) give diagonals D(t_j-t_i) (top half)
# and D(u_j-u_i) (bottom half); together exactly {1..32}.
T_ROT = [0, 8, 22, 24, 42, 48, 49, 57, 60]
U_ROT = [0, 59, 16, 38, 55, 22, 30, 54, 35]
OPS = [(1, 3), (2, 3), (1, 4), (2, 4), (3, 4), (4, 5), (1, 6), (2, 6),
       (6, 7), (0, 7), (4, 7), (5, 7), (2, 8), (3, 8), (5, 8), (6, 8)]
NB = len(T_ROT)        # 9 rotation buffers
KCH = 1 + len(OPS)     # 17 contraction chunks of 128 rows
PASS_FD = [256, 512, 1024, 1024, 1024, 256]
assert sum(PASS_FD) == PIX
NPASS = len(PASS_FD)
# chunk order within a pass: chunk 0 (ScalarE path, gated on the 1.3us
# activation-table load) goes last so TT-produced chunks start the
# accumulation immediately.
KORDER = list(range(1, KCH)) + [0]
# packed order of the rotation buffers = first-use order under KORDER,
# so pass 0 can be split into two DMAs with the urgent half first.
BUF_ORDER = [1, 3, 2, 4, 5, 6, 7, 0, 8]
POS = {b: p for p, b in enumerate(BUF_ORDER)}
NSPLIT = 5  # pass-0 buffers in the first (urgent) DMA

F32 = mybir.dt.float32
BF16 = mybir.dt.bfloat16


def row_pairs():
    """Channel pair (c1, c2) for every global y row, or ('lin', c)."""
    rows = []
    for p in range(128):  # chunk 0
        rows.append(("lin", p) if p < 64 else (p - 64, p - 64))
    for (i, j) in OPS:
        for p in range(128):
            if p < 64:
                c1, c2 = (p + T_ROT[i]) % 64, (p + T_ROT[j]) % 64
            else:
                c1, c2 = (p - 64 + U_ROT[i]) % 64, (p - 64 + U_ROT[j]) % 64
            rows.append((min(c1, c2), max(c1, c2)))
    return rows


def build_wt(fc_w):
    """Permute fc_w [OUT, 2144] into Wt [KCH, 128, OUT] matching y rows."""
    ii, jj = np.triu_indices(C)
    pair2col = {(a, b): C + k for k, (a, b) in enumerate(zip(ii, jj))}
    wt = np.zeros((KCH * 128, OUT), np.float32)
    seen = set()
    for g, r in enumerate(row_pairs()):
        if r[0] == "lin":
            wt[g] = fc_w[:, r[1]]
        elif r not in seen:
            seen.add(r)
            wt[g] = fc_w[:, pair2col[r]]
    assert len(seen) == C * (C + 1) // 2
    return np.ascontiguousarray(wt.reshape(KCH, 128, OUT))


_cached = None


def _build_module():
    global _cached
    if _cached is not None:
        return _cached
    nc = bacc.Bacc("TRN2", target_bir_lowering=False, debug=False,
                   num_devices=NCORES)
    # all 9 rotation buffers, pass-blocked + packed in BUF_ORDER:
    # pass p occupies columns [9*off_p, 9*off_p + 9*FD_p), buffer j (packed)
    # at [9*off_p + j*FD_p, 9*off_p + (j+1)*FD_p).
    ball_d = nc.dram_tensor("ball", [128, NB * PIX], BF16,
                            kind="ExternalInput")
    # weight matrix, partition-major so DMA rows are contiguous
    wt_d = nc.dram_tensor("wt", [128, KCH * OUT], BF16, kind="ExternalInput")
    out_d = nc.dram_tensor("out", [128, 2, PIX], BF16, kind="ExternalOutput")

    with tile.TileContext(nc) as tc:
        with tc.tile_pool(name="wt", bufs=1) as wt_pool, \
             tc.tile_pool(name="bsrc", bufs=2) as b_pool, \
             tc.tile_pool(name="y", bufs=8) as y_pool, \
             tc.tile_pool(name="ostage", bufs=3) as o_pool, \
             tc.tile_pool(name="psum", bufs=8, space="PSUM") as ps_pool:

            wt_t = wt_pool.tile([128, KCH * OUT], BF16, name="wtt")

            # PE warm-up: junk matmuls on a memset tile keep TensorE
            # CONTINUOUSLY busy from ~5us (right after the framework
            # preamble) until the first real inputs land (~12us), so the
            # HAM clock-gate reaches 8/8 and the real stream starts warm.
            # A sparse cold start is worse than a late dense one: cold
            # matmuls run at 1.2GHz and idle gaps keep resetting HAM's
            # 3.4us busy window.
            wu = wt_pool.tile([128, 640], BF16, name="warmup")
            nc.vector.memset(wu[:, :], 0.0)
            ps_w = ps_pool.tile([128, 512], F32, tag="ps", name="warm_ps")
            for r in range(8):
                nc.tensor.matmul(ps_w[:, :], wu[:, 0:128], wu[:, 128:640],
                                 start=True, stop=True)

            off = 0
            for ps, FD in enumerate(PASS_FD):
                NT = max(1, FD // 512)
                NW = min(512, FD)  # matmul free width
                blk = NB * off     # start column of this pass's block
                ball_t = b_pool.tile([128, NB * 1024], BF16, tag="ball",
                                     name=f"ball_{ps}")
                if ps == 0:
                    # Each dma_start gets its OWN hw queue (~180GB/s each)
                    # and transfers run in parallel even when issued from
                    # one engine; only the ~0.57us/issue sequencer time
                    # serializes. Split the head loads finely, most-urgent
                    # first. GpSimd's SWDGE queue starts ~3.5us later than
                    # the HWDGE ones, so it only gets weights needed
                    # mid-pass; Scalar's DMAs precede all its compute so
                    # they can't block on drains.
                    nc.scalar.dma_start(wt_t[:, 3 * OUT:9 * OUT],
                                        wt_d.ap()[:, 3 * OUT:9 * OUT])
                    nc.gpsimd.dma_start(wt_t[:, 9 * OUT:13 * OUT],
                                        wt_d.ap()[:, 9 * OUT:13 * OUT])
                    nc.sync.dma_start(ball_t[:, :2 * FD],
                                      ball_d.ap()[:, blk:blk + 2 * FD])
                    nc.sync.dma_start(
                        ball_t[:, 2 * FD:NSPLIT * FD],
                        ball_d.ap()[:, blk + 2 * FD:blk + NSPLIT * FD])
                    nc.sync.dma_start(
                        ball_t[:, NSPLIT * FD:NB * FD],
                        ball_d.ap()[:, blk + NSPLIT * FD:blk + NB * FD])
                    nc.sync.dma_start(wt_t[:, OUT:3 * OUT],
                                      wt_d.ap()[:, OUT:3 * OUT])
                    nc.sync.dma_start(wt_t[:, 13 * OUT:],
                                      wt_d.ap()[:, 13 * OUT:])
                    nc.sync.dma_start(wt_t[:, :OUT], wt_d.ap()[:, :OUT])
                elif ps == 1:
                    # pass 1 arrives while pass 0 is still streaming: give
                    # its urgent half two parallel queues as well
                    nc.sync.dma_start(ball_t[:, :2 * FD],
                                      ball_d.ap()[:, blk:blk + 2 * FD])
                    nc.sync.dma_start(
                        ball_t[:, 2 * FD:NSPLIT * FD],
                        ball_d.ap()[:, blk + 2 * FD:blk + NSPLIT * FD])
                    nc.gpsimd.dma_start(
                        ball_t[:, NSPLIT * FD:NB * FD],
                        ball_d.ap()[:, blk + NSPLIT * FD:blk + NB * FD])
                else:
                    # split every pass across two DMA queues so neither
                    # (~180GB/s each) gates a pass boundary
                    nc.sync.dma_start(
                        ball_t[:, :NSPLIT * FD],
                        ball_d.ap()[:, blk:blk + NSPLIT * FD])
                    nc.gpsimd.dma_start(
                        ball_t[:, NSPLIT * FD:NB * FD],
                        ball_d.ap()[:, blk + NSPLIT * FD:blk + NB * FD])

                def bsl(i):
                    return ball_t[:, POS[i] * FD:(POS[i] + 1) * FD]

                psum = [ps_pool.tile([128, 512], F32, tag="ps",
                                     name=f"ps{ps}_{g}")
                        for g in range(2 * NT)]

                for k in KORDER:
                    yk = y_pool.tile([128, 1024], BF16, tag="y",
                                     name=f"y{ps}_{k}")
                    if k == 0:
                        # linear rows + squares from the resident b0 slice
                        b0 = bsl(0)
                        nc.scalar.activation(
                            yk[0:64, :FD], b0[0:64, :],
                            mybir.ActivationFunctionType.Identity)
                        nc.scalar.activation(
                            yk[64:128, :FD], b0[64:128, :],
                            mybir.ActivationFunctionType.Square)
                    else:
                        i, j = OPS[k - 1]
                        nc.vector.tensor_mul(yk[:, :FD], bsl(i), bsl(j))
                    for m in range(2):
                        lhsT = wt_t[:, k * OUT + m * 128:k * OUT + (m + 1) * 128]
                        for n in range(NT):
                            nc.tensor.matmul(
                                psum[m * NT + n][:, :NW],
                                lhsT,
                                yk[:, n * NW:(n + 1) * NW],
                                start=(k == KORDER[0]), stop=(k == KORDER[-1]))

                last = ps == NPASS - 1
                ot = o_pool.tile([128, 2048], BF16, tag="ostage",
                                 name=f"o{ps}")
                for m in range(2):
                    for n in range(NT):
                        src = psum[m * NT + n][:, :NW]
                        dst = ot[:, m * FD + n * NW:m * FD + (n + 1) * NW]
                        if last and m == 1:
                            # tail: drain half the PSUM on the idle VectorE
                            nc.vector.tensor_copy(dst, src)
                        else:
                            nc.scalar.activation(
                                dst, src, mybir.ActivationFunctionType.Identity)
                eng = nc.sync if last else nc.scalar
                eng.dma_start(out_d.ap()[:, :, off:off + FD], ot[:, :2 * FD])
                off += FD
    nc.compile()
    _cached = nc
    return nc


def make_in_maps(x, wt):
    # [KCH, 128, OUT] -> [128, KCH*OUT], bf16
    wtp = np.ascontiguousarray(
        wt.transpose(1, 0, 2).reshape(128, KCH * OUT)).astype(ml_dtypes.bfloat16)
    in_maps = []
    for b in range(B):
        xc = np.ascontiguousarray(
            np.asarray(x[b], np.float32).reshape(C, PIX)).astype(ml_dtypes.bfloat16)
        bufs = [np.concatenate(
            [np.roll(xc, -T_ROT[i], axis=0), np.roll(xc, -U_ROT[i], axis=0)])
            for i in range(NB)]
        ball = np.empty((128, NB * PIX), ml_dtypes.bfloat16)
        o = 0
        for FD in PASS_FD:
            for j, bi in enumerate(BUF_ORDER):
                ball[:, NB * o + j * FD:NB * o + (j + 1) * FD] = \
                    bufs[bi][:, o:o + FD]
            o += FD
        in_maps.append({"wt": wtp, "ball": ball})
    return in_maps


def assemble_out(res):
    outs = []
    for b in range(B):
        o = np.asarray(res.results[b]["out"]).astype(np.float32)  # [128,2,PIX]
        outs.append(o.transpose(1, 0, 2).reshape(OUT, H, W))
    return np.stack(outs)


def kernel(x, fc_w):
    x = np.asarray(x, dtype=np.float32)
    fc_w = np.asarray(fc_w, dtype=np.float32)
    nc = _build_module()
    wt = build_wt(fc_w)
    res = run_bass_kernel_spmd(nc, make_in_maps(x, wt), list(range(NCORES)))
    return assemble_out(res)


# revision 24
# speedup vs baseline: 1.1746x; 1.0501x over previous
"""Trainium2 kernel for ChannelQuadLayer.

Per-pixel quadratic channel expansion + 1x1 conv:
    quad = x[:, ii] * x[:, jj]  (all 2080 upper-tri channel pairs)
    y    = concat([x, quad])    -> [B, 2144, H, W]
    out  = einsum('bchw,oc->bohw', y, fc_w)

Strategy (8 NeuronCores, batch-parallel, one sample per core):
  * The 2080 unordered channel pairs are exactly the cyclic diagonals
    d=0..32 of the 64-channel index ring: pairs {i, (i+d)%64}.
  * Host prepares 9 "rotation buffers" B_k = [roll(x,-t_k); roll(x,-u_k)]
    (128 partitions x 4096 pixels, bfloat16). A single elementwise
    multiply of two such buffers yields TWO complete cyclic diagonals
    (top half: diagonal t_j - t_i, bottom half: u_j - u_i). A difference
    cover produces all diagonals 1..32 in 16 multiplies; diagonal 0
    (squares) comes from one ScalarE Square op.
  * y-rows: 64 linear + 64 squares + 16*128 pair rows = 2176 = 17*128,
    an exact 17-chunk contraction. fc_w is permuted/padded to this row
    order on the host (duplicate pair rows get zero weight).
  * GEMM: out[256, 4096] = Wt[2176, 256]^T @ y[2176, 4096] on TensorE
    in bfloat16 (rel err ~3.5e-3, well under the 2e-2 gate; the bf16
    weight dtype enables the compiler's Fast Weight Load so the
    128-cycle LDWEIGHTS no longer serializes the matmul stream, unlike
    float32r), accumulating 17 chunks into fp32 PSUM, k-outer so each
    y chunk is consumed right after its producer. bf16 also gives the
    DVE 2x perf mode for the pair products and halves all DMA.
  * All 9 rotation buffers are packed into ONE DRAM tensor in
    pass-blocked, first-use order, so each pixel pass is a single
    dma_start on SyncE (DMA issues cost ~0.6us of sequencer time each
    and, queued behind compute, caused PE starvation). Outputs
    similarly drain to one staging tile and one DMA per pass.
  * Output leaves the chip as bf16 [128, 2, PIX]; the host upcasts and
    reorders - host work is untimed.
  * Pixel passes [256, 512, 1024, 1024, 1024, 256]: small first pass
    fills the pipeline fast, small last pass shortens the tail. Chunk 0
    (ScalarE, gated on the 1.3us activation-table load) is reordered to
    the END of each pass's accumulation.
  * A few junk matmuls on a memset tile warm the PE clock gate (HAM)
    while the first DMAs are in flight.
"""

import sys

sys.path.insert(0, "/opt/trn_rl_repo")

import numpy as np
import ml_dtypes

import concourse.bass as bass
import concourse.tile as tile
from concourse import bacc, mybir
from concourse.bass_utils import run_bass_kernel_spmd

B, C, H, W = 8, 64, 64, 64
PIX = H * W  # 4096
OUT = 256
NCORES = 8

# rotation difference cover: ops (i,j) give diagonals D(t_j-t_i) (top half)
# and D(u_j-u_i) (bottom half); together exactly {1..32}.
T_ROT = [0, 8, 22, 24, 42, 48, 49, 57, 60]
U_ROT = [0, 59, 16, 38, 55, 22, 30, 54, 35]
OPS = [(1, 3), (2, 3), (1, 4), (2, 4), (3, 4), (4, 5), (1, 6), (2, 6),
       (6, 7), (0, 7), (4, 7), (5, 7), (2, 8), (3, 8), (5, 8), (6, 8)]
NB = len(T_ROT)        # 9 rotation buffers
KCH = 1 + len(OPS)     # 17 contraction chunks of 128 rows
PASS_FD = [256, 512, 1024, 1024, 1024, 256]
assert sum(PASS_FD) == PIX
NPASS = len(PASS_FD)
# chunk order within a pass: chunk 0 (ScalarE path, gated on the 1.3us
# activation-table load) goes last so TT-produced chunks start the
# accumulation immediately.
KORDER = list(range(1, KCH)) + [0]
# packed order of the rotation buffers = first-use order under KORDER,
# so pass 0 can be split into two DMAs with the urgent half first.
BUF_ORDER = [1, 3, 2, 4, 5, 6, 7, 0, 8]
POS = {b: p for p, b in enumerate(BUF_ORDER)}
NSPLIT = 5  # pass-0 buffers in the first (urgent) DMA

F32 = mybir.dt.float32
BF16 = mybir.dt.bfloat16


def row_pairs():
    """Channel pair (c1, c2) for every global y row, or ('lin', c)."""
    rows = []
    for p in range(128):  # chunk 0
        rows.append(("lin", p) if p < 64 else (p - 64, p - 64))
    for (i, j) in OPS:
        for p in range(128):
            if p < 64:
                c1, c2 = (p + T_ROT[i]) % 64, (p + T_ROT[j]) % 64
            else:
                c1, c2 = (p - 64 + U_ROT[i]) % 64, (p - 64 + U_ROT[j]) % 64
            rows.append((min(c1, c2), max(c1, c2)))
    return rows


def build_wt(fc_w):
    """Permute fc_w [OUT, 2144] into Wt [KCH, 128, OUT] matching y rows."""
    ii, jj = np.triu_indices(C)
    pair2col = {(a, b): C + k for k, (a, b) in enumerate(zip(ii, jj))}
    wt = np.zeros((KCH * 128, OUT), np.float32)
    seen = set()
    for g, r in enumerate(row_pairs()):
        if r[0] == "lin":
            wt[g] = fc_w[:, r[1]]
        elif r not in seen:
            seen.add(r)
            wt[g] = fc_w[:, pair2col[r]]
    assert len(seen) == C * (C + 1) // 2
    return np.ascontiguousarray(wt.reshape(KCH, 128, OUT))


_cached = None


def _build_module():
    global _cached
    if _cached is not None:
        return _cached
    nc = bacc.Bacc("TRN2", target_bir_lowering=False, debug=False,
                   num_devices=NCORES)
    # all 9 rotation buffers, pass-blocked + packed in BUF_ORDER:
    # pass p occupies columns [9*off_p, 9*off_p + 9*FD_p), buffer j (packed)
    # at [9*off_p + j*FD_p, 9*off_p + (j+1)*FD_p).
    ball_d = nc.dram_tensor("ball", [128, NB * PIX], BF16,
                            kind="ExternalInput")
    # weight matrix, partition-major so DMA rows are contiguous
    wt_d = nc.dram_tensor("wt", [128, KCH * OUT], BF16, kind="ExternalInput")
    out_d = nc.dram_tensor("out", [128, 2, PIX], BF16, kind="ExternalOutput")

    with tile.TileContext(nc) as tc:
        with tc.tile_pool(name="wt", bufs=1) as wt_pool, \
             tc.tile_pool(name="bsrc", bufs=2) as b_pool, \
             tc.tile_pool(name="y", bufs=8) as y_pool, \
             tc.tile_pool(name="ostage", bufs=3) as o_pool, \
             tc.tile_pool(name="psum", bufs=8, space="PSUM") as ps_pool:

            wt_t = wt_pool.tile([128, KCH * OUT], BF16, name="wtt")

            # PE warm-up: junk matmuls on a memset tile keep TensorE
            # CONTINUOUSLY busy from ~5us (right after the framework
            # preamble) until the first real inputs land (~12us), so the
            # HAM clock-gate reaches 8/8 and the real stream starts warm.
            # A sparse cold start is worse than a late dense one: cold
            # matmuls run at 1.2GHz and idle gaps keep resetting HAM's
            # 3.4us busy window.
            wu = wt_pool.tile([128, 640], BF16, name="warmup")
            nc.vector.memset(wu[:, :], 0.0)
            ps_w = ps_pool.tile([128, 512], F32, tag="ps", name="warm_ps")
            for r in range(9):
                nc.tensor.matmul(ps_w[:, :], wu[:, 0:128], wu[:, 128:640],
                                 start=True, stop=True)

            off = 0
            for ps, FD in enumerate(PASS_FD):
                NT = max(1, FD // 512)
                NW = min(512, FD)  # matmul free width
                blk = NB * off     # start column of this pass's block
                ball_t = b_pool.tile([128, NB * 1024], BF16, tag="ball",
                                     name=f"ball_{ps}")
                if ps == 0:
                    # DMA queues are PER-ENGINE (~180GB/s each): balance the
                    # ~1.7MB head load across the three DMA-capable engines,
                    # most-urgent first per queue. GpSimd's SWDGE queue
                    # starts ~3.5us later than the HWDGE ones, so it only
                    # gets weights needed mid-pass; Scalar's DMAs precede
                    # all its compute so they can't block on drains.
                    nc.scalar.dma_start(wt_t[:, OUT:9 * OUT],
                                        wt_d.ap()[:, OUT:9 * OUT])
                    nc.gpsimd.dma_start(wt_t[:, 9 * OUT:13 * OUT],
                                        wt_d.ap()[:, 9 * OUT:13 * OUT])
                    nc.sync.dma_start(
                        ball_t[:, :NSPLIT * FD],
                        ball_d.ap()[:, blk:blk + NSPLIT * FD])
                    nc.sync.dma_start(
                        ball_t[:, NSPLIT * FD:NB * FD],
                        ball_d.ap()[:, blk + NSPLIT * FD:blk + NB * FD])
                    nc.sync.dma_start(wt_t[:, 13 * OUT:],
                                      wt_d.ap()[:, 13 * OUT:])
                    nc.sync.dma_start(wt_t[:, :OUT], wt_d.ap()[:, :OUT])
                else:
                    # split every pass across two DMA queues so neither
                    # (~180GB/s each) gates a pass boundary
                    nc.sync.dma_start(
                        ball_t[:, :NSPLIT * FD],
                        ball_d.ap()[:, blk:blk + NSPLIT * FD])
                    nc.gpsimd.dma_start(
                        ball_t[:, NSPLIT * FD:NB * FD],
                        ball_d.ap()[:, blk + NSPLIT * FD:blk + NB * FD])

                def bsl(i):
                    return ball_t[:, POS[i] * FD:(POS[i] + 1) * FD]

                psum = [ps_pool.tile([128, 512], F32, tag="ps",
                                     name=f"ps{ps}_{g}")
                        for g in range(2 * NT)]

                for k in KORDER:
                    yk = y_pool.tile([128, 1024], BF16, tag="y",
                                     name=f"y{ps}_{k}")
                    if k == 0:
                        # linear rows + squares from the resident b0 slice
                        b0 = bsl(0)
                        nc.scalar.activation(
                            yk[0:64, :FD], b0[0:64, :],
                            mybir.ActivationFunctionType.Identity)
                        nc.scalar.activation(
                            yk[64:128, :FD], b0[64:128, :],
                            mybir.ActivationFunctionType.Square)
                    else:
                        i, j = OPS[k - 1]
                        nc.vector.tensor_mul(yk[:, :FD], bsl(i), bsl(j))
                    for m in range(2):
                        lhsT = wt_t[:, k * OUT + m * 128:k * OUT + (m + 1) * 128]
                        for n in range(NT):
                            nc.tensor.matmul(
                                psum[m * NT + n][:, :NW],
                                lhsT,
                                yk[:, n * NW:(n + 1) * NW],
                                start=(k == KORDER[0]), stop=(k == KORDER[-1]))

                last = ps == NPASS - 1
                ot = o_pool.tile([128, 2048], BF16, tag="ostage",
                                 name=f"o{ps}")
                for m in range(2):
                    for n in range(NT):
                        src = psum[m * NT + n][:, :NW]
                        dst = ot[:, m * FD + n * NW:m * FD + (n + 1) * NW]
                        if last and m == 1:
                            # tail: drain half the PSUM on the idle VectorE
                            nc.vector.tensor_copy(dst, src)
                        else:
                            nc.scalar.activation(
                                dst, src, mybir.ActivationFunctionType.Identity)
                eng = nc.sync if last else nc.scalar
                eng.dma_start(out_d.ap()[:, :, off:off + FD], ot[:, :2 * FD])
                off += FD
    nc.compile()
    _cached = nc
    return nc


def make_in_maps(x, wt):
    # [KCH, 128, OUT] -> [128, KCH*OUT], bf16
    wtp = np.ascontiguousarray(
        wt.transpose(1, 0, 2).reshape(128, KCH * OUT)).astype(ml_dtypes.bfloat16)
    in_maps = []
    for b in range(B):
        xc = np.ascontiguousarray(
            np.asarray(x[b], np.float32).reshape(C, PIX)).astype(ml_dtypes.bfloat16)
        bufs = [np.concatenate(
            [np.roll(xc, -T_ROT[i], axis=0), np.roll(xc, -U_ROT[i], axis=0)])
            for i in range(NB)]
        ball = np.empty((128, NB * PIX), ml_dtypes.bfloat16)
        o = 0
        for FD in PASS_FD:
            for j, bi in enumerate(BUF_ORDER):
                ball[:, NB * o + j * FD:NB * o + (j + 1) * FD] = \
                    bufs[bi][:, o:o + FD]
            o += FD
        in_maps.append({"wt": wtp, "ball": ball})
    return in_maps


def assemble_out(res):
    outs = []
    for b in range(B):
        o = np.asarray(res.results[b]["out"]).astype(np.float32)  # [128,2,PIX]
        outs.append(o.transpose(1, 0, 2).reshape(OUT, H, W))
    return np.stack(outs)


def kernel(x, fc_w):
    x = np.asarray(x, dtype=np.float32)
    fc_w = np.asarray(fc_w, dtype=np.float32)
    nc = _build_module()
    wt = build_wt(fc_w)
    res = run_bass_kernel_spmd(nc, make_in_maps(x, wt), list(range(NCORES)))
    return assemble_out(res)
